# revision 1
# baseline (speedup 1.0000x reference)
"""Trainium2 Bass kernel for NodeCorrespondenceSelector (topk_masking).

Reference semantics: mask confidence <= 0.1 to zero, take the 256 SMALLEST
of the masked [B, N*M] map (top_k of the negation), unravel to (src, tgt).

Key property: ~10% of uniform entries are <= 0.1 and become exactly 0.0,
so the 256 smallest masked values are all 0.0 and XLA's stable top_k picks
them in ascending flat-index order.  The answer is therefore exactly the
first 256 flat indices with value <= 0.1 per batch row, ascending.  Those
all live in a short prefix of each row (the 256th hit sits near flat
position ~2560), so each core only needs to scan a 3584-element prefix.
The host verifies the device result is consistent (>= 256 hits in the
prefix, strictly increasing positions) and falls back to an exact host
computation otherwise (P(<256 hits in 3584) ~ 4e-9 per row).

Device algorithm per core (one batch row per core, 8 cores):
  1. mask m = (x <= 0.1) on a [128, 32] tile (flat order, partition-major)
  2. L = inclusive cumsum of m along the free dim (tensor_tensor_scan)
  3. per-partition totals t -> flat [1,129] -> inclusive scan of
     [0, t0..t127] = exclusive base offsets
  4. broadcast C(i) = L(i) + base(p(i)) to all 128 partitions via two
     accumulating rank-1 bf16 matmuls per 512-wide PSUM bank:
       ps[q, i] = 1*Lf[i] + 1*base[i/32]
     (bf16 rounding is safe: C values that matter for k <= 255 are < 256
     and exact in bf16; rounded larger values can never fall below 256)
  5. counts: out[k] = sum_i [C(i) <= k]  (= flat position of (k+1)-th hit)
       k =   0..127: VectorE  tensor_scalar(is_le, accum_out)
       k = 128..255: ScalarE  activation(Relu, bias=k+1, scale=-1,
                     accum_out) giving A(k) = sum_i relu(k+1 - C(i));
                     host takes adjacent differences (A(127) = sum of
                     the VectorE half).
     PSUM is split in two halves read in opposite order by the two
     engines so they run concurrently (same-tile reads serialize).
"""

import numpy as np

_THRES = np.float32(0.1)
_K = 256
_P = 128            # SBUF partitions
_F = 28             # free elements per partition in the prefix tile
_P2 = _P * _F       # 3584: prefix elements scanned on device per row
_H = _P2 // 2       # half width (one PSUM tile)
_NCORES = 8

_NC_CACHE = {}


def _build_nc():
    import concourse.bacc as bacc
    import concourse.mybir as mybir
    from concourse.tile import TileContext

    dt = mybir.dt
    op = mybir.AluOpType
    act = mybir.ActivationFunctionType

    nc = bacc.Bacc(trn_type="TRN2", debug=False, enable_asserts=False)
    x = nc.dram_tensor("x", [_P, _F], dt.float32, kind="ExternalInput")
    kvec = nc.dram_tensor("kvec", [_P, 2], dt.float32, kind="ExternalInput")
    tri = nc.dram_tensor("tri", [_P, _P], dt.bfloat16, kind="ExternalInput")
    cnt = nc.dram_tensor("cnt", [_P, 8], dt.float32, kind="ExternalOutput")

    with TileContext(nc) as tc:
        with (
            tc.tile_pool(name="sb", bufs=1) as pool,
            tc.tile_pool(name="ps", bufs=1, space="PSUM") as psum,
        ):
            xt = pool.tile([_P, _F], dt.float32, tag="xt")
            nc.sync.dma_start(xt[:], x[:, :])
            trit = pool.tile([_P, _P], dt.bfloat16, tag="trit")
            nc.scalar.dma_start(trit[:], tri[:, :])
            kv = pool.tile([_P, 2], dt.float32, tag="kv")
            nc.scalar.dma_start(kv[:], kvec[:, :])

            o2 = pool.tile([2, _P], dt.bfloat16, tag="o2")
            nc.vector.memset(o2[:2, :], 1.0)
            z = pool.tile([_P, _F], dt.float32, tag="z")
            nc.vector.memset(z[:], 0.0)

            m = pool.tile([_P, _F], dt.float32, tag="m")
            nc.vector.tensor_scalar(m[:], xt[:], float(_THRES), None, op.is_le)
            t = pool.tile([_P, 1], dt.bfloat16, tag="t")
            with nc.allow_low_precision(reason="counts <= 28 are exact in bf16"):
                nc.vector.tensor_reduce(
                    t[:], m[:], axis=mybir.AxisListType.X, op=op.add
                )
            L = pool.tile([_P, _F], dt.bfloat16, tag="L")
            nc.vector.tensor_tensor_scan(
                L[:], m[:], z[:], 0.0, op.add, op.add
            )

            # four PSUM tiles (PSUM dependency tracking is tile-granular,
            # so independent readers need separate tiles): 2+2+2+1 banks
            psA = psum.tile([_P, 1024], dt.float32, tag="psA")
            psB = psum.tile([_P, 1024], dt.float32, tag="psB")
            psC = psum.tile([_P, 1024], dt.float32, tag="psC")
            psD = psum.tile([_P, 512], dt.float32, tag="psD")
            psT = [psA, psB, psC, psD]
            psW = [1024, 1024, 1024, 512]

            # base[q] = sum_{p<q} t[p] via PE: lhsT = tri (tri[p, q] = 1 iff
            # p < q), rhs = t; lands in psA bank 0, which the broadcast
            # matmuls below overwrite afterwards.
            nc.tensor.matmul(
                psT[0][:, 0:1], trit[:], t[:], start=True, stop=True
            )
            # expand base to [128, _F] (per-partition broadcast)
            brep = pool.tile([_P, _F], dt.bfloat16, tag="brep")
            nc.vector.tensor_scalar(brep[:], z[:], psT[0][:, 0:1], None, op.add)

            # T2 row 0 = L flattened (partition-major) = L(i); row 1 = brep
            # flattened = base[i // 32]
            T2 = pool.tile([2, _P2], dt.bfloat16, tag="T2")
            nc.sync.dma_start(T2[:1, :], L[:])
            nc.scalar.dma_start(T2[1:2, :], brep[:])

            # ps[q, i] = Lf[i] + base[i // 32]  for all q
            npb = 512  # one PSUM bank of f32
            for b in range(_P2 // npb):
                sl = slice(b * npb, (b + 1) * npb)
                pst = psT[min(b // 2, 3)]
                off = b * npb - [0, 1024, 2048, 3072][min(b // 2, 3)]
                nc.tensor.matmul(
                    pst[:, off : off + npb], o2[:2, :], T2[:2, sl],
                    start=True, stop=True,
                )

            # counts over four quarters, engines staggered so both run
            # concurrently and start as soon as the relevant banks are done
            # (disjoint-slice reads don't serialize; same-slice reads do,
            # in emission order)
            G0 = pool.tile([_P, _P2], dt.float32, tag="G0")
            G1 = pool.tile([_P, _P2], dt.float32, tag="G1")
            S = pool.tile([_P, 8], dt.float32, tag="S")

            # DVE first-reads tiles 0 and 2; ACT first-reads tiles 1 and 3;
            # second round swapped, so both engines run concurrently.
            GOFF = [0, 1024, 2048, 3072]
            for eng, ti, col in (
                ("v", 0, 0), ("a", 1, 5), ("v", 2, 1), ("a", 3, 7),
                ("v", 1, 2), ("a", 0, 4), ("v", 3, 3), ("a", 2, 6),
            ):
                pst = psT[ti]
                w = psW[ti]
                sl = slice(GOFF[ti], GOFF[ti] + w)
                if eng == "v":
                    nc.vector.tensor_scalar(
                        G0[:, sl], pst[:, :], kv[:, 0:1], None,
                        op.is_le, op1=op.add, accum_out=S[:, col : col + 1],
                    )
                else:
                    nc.scalar.activation(
                        G1[:, sl], pst[:, :], act.Relu,
                        bias=kv[:, 1:2], scale=-1.0,
                        accum_out=S[:, col : col + 1],
                    )
            nc.sync.dma_start(cnt[:, :], S[:])
    nc.compile()
    return nc


def _get_nc():
    if "nc" not in _NC_CACHE:
        _NC_CACHE["nc"] = _build_nc()
    return _NC_CACHE["nc"]


def _make_kvec():
    # col 0: k values 0..127 for the VectorE is_le half
    # col 1: Relu biases k+1 = 129..256 for the ScalarE half (k = 128..255)
    kvec = np.empty((_P, 2), np.float32)
    kvec[:, 0] = np.arange(_P, dtype=np.float32)
    kvec[:, 1] = np.arange(_P, dtype=np.float32) + 129.0
    return kvec


def _decode_counts(cnt_out):
    """cnt_out: [128, 8] f32 from one core -> [256] int64 positions."""
    s0 = cnt_out[:, 0:4].astype(np.float64).sum(axis=1)
    s1 = cnt_out[:, 4:8].astype(np.float64).sum(axis=1)
    a_prev = np.concatenate([[s0.sum()], s1[:-1]])  # A(127..254)
    hi = s1 - a_prev
    return np.concatenate([s0, hi]).astype(np.int64)


def _run_device(prefix, trace=False):
    """prefix: [8, 4096] f32.  Returns (positions [8, 256] int64, results)."""
    import ml_dtypes
    from concourse.bass_utils import run_bass_kernel_spmd

    nc = _get_nc()
    kvec = _make_kvec()
    tri = np.triu(np.ones((_P, _P), np.float32), 1).astype(ml_dtypes.bfloat16)
    in_maps = [
        {
            "x": np.ascontiguousarray(prefix[c].reshape(_P, _F)),
            "kvec": kvec,
            "tri": tri,
        }
        for c in range(_NCORES)
    ]
    res = run_bass_kernel_spmd(
        nc, in_maps, core_ids=list(range(_NCORES)), trace=trace
    )
    pos = np.stack([_decode_counts(res.results[c]["cnt"]) for c in range(_NCORES)])
    return pos, res


def _host_row(flat_row):
    """Exact reference semantics for one row (fallback path)."""
    mask = flat_row <= _THRES
    hits = np.flatnonzero(mask)
    if hits.size >= _K:
        return hits[:_K].astype(np.int64)
    masked = np.where(flat_row > _THRES, flat_row, np.float32(0.0))
    order = np.argsort(masked, kind="stable")
    return order[:_K].astype(np.int64)


def kernel(confidence_map):
    cm = np.asarray(confidence_map)
    if cm.dtype != np.float32:
        cm = cm.astype(np.float32)
    B = cm.shape[0]
    num_tgt = cm.shape[2]
    flat = cm.reshape(B, -1)

    idx = None
    if B == _NCORES and flat.shape[1] >= _P2:
        pos, _ = _run_device(flat[:, :_P2])
        ok = bool(
            pos.min() >= 0
            and pos.max() < _P2
            and np.all(np.diff(pos, axis=1) > 0)
        )
        if ok:
            idx = pos
    if idx is None:
        idx = np.stack([_host_row(flat[b]) for b in range(B)])

    src = (idx // num_tgt).astype(np.int32)
    tgt = (idx % num_tgt).astype(np.int32)
    return np.stack([src, tgt], axis=-1)



# revision 2
# speedup vs baseline: 1.3441x; 1.3441x over previous
"""Trainium2 Bass kernel for NodeCorrespondenceSelector (topk_masking), v2.

Reference semantics: mask confidence <= 0.1 to zero, take the 256 SMALLEST
of the masked [B, N*M] map (top_k of the negation), unravel to (src, tgt).
~10% of uniform entries are <= 0.1 and become exactly 0.0, so the answer is
the first 256 flat indices with value <= 0.1 per batch row, ascending; all
of them live in a short prefix (256th hit ~ flat position 2600), so each
core scans a 3584-element prefix laid out [128 partitions x 28].

v2 device algorithm (per core, one batch row; everything stays in the
native [128, 28] layout -- no flatten DMAs, no matmuls, no PSUM):
  1. m = (x <= 0.1)                               [128, 28]
  2. L = inclusive cumsum of m along free dim     [128, 28]
  3. G[p, j, f] = (L[p, f] < j+1)  for j in 0..J-1, via stride-0
     broadcast APs (L broadcast along j, iota-by-scan jc broadcast
     along f)                                     [128, J*28]
  4. CDF[p, j] = sum_f G[p, j, f]  (tensor_reduce over the innermost
     axis of the 3D view)                         [128, J]
CDF[p, j] is the in-partition position of the (j+1)-th hit when
j < t[p] (t[p] = hits in partition p), else 28.  The host recovers
t[p] = #{j : CDF[p,j] < 28} (exact whenever t[p] < J, detectable
otherwise), builds the hit-count prefix sum over partitions, and decodes
rank r -> partition p(r) + local position CDF[p(r), r - base[p(r)]].
The host verifies the decode is consistent (integral CDF, nondecreasing
rows, t < J, >= 256 hits, strictly increasing positions) and falls back
to an exact host computation otherwise.
"""

import numpy as np

_THRES = np.float32(0.1)
_K = 256
_P = 128            # SBUF partitions
_F = 28             # free elements per partition in the prefix tile
_PRE = _P * _F      # 3584: prefix elements scanned on device per row
_J = 16             # CDF thresholds per partition (max decodable hits/partition)
_NCORES = 8

_NC_CACHE = {}


def _build_nc():
    import concourse.bacc as bacc
    import concourse.mybir as mybir
    from concourse.tile import TileContext

    dt = mybir.dt
    op = mybir.AluOpType

    nc = bacc.Bacc(trn_type="TRN2", debug=False, enable_asserts=False)
    x = nc.dram_tensor("x", [_P, _F], dt.float32, kind="ExternalInput")
    cnt = nc.dram_tensor("cnt", [_P, _J], dt.bfloat16, kind="ExternalOutput")

    with TileContext(nc) as tc:
        with tc.tile_pool(name="sb", bufs=1) as pool:
            xt = pool.tile([_P, _F], dt.float32, tag="xt")
            # input DMA split by partition halves across two queues
            nc.sync.dma_start(xt[0:64, :], x[0:64, :])
            nc.scalar.dma_start(xt[64:128, :], x[64:128, :])

            # constants built on DVE before the input lands
            z = pool.tile([_P, _F], dt.float32, tag="z")
            nc.vector.memset(z[:], 0.0)
            o16 = pool.tile([_P, _J], dt.float32, tag="o16")
            nc.vector.memset(o16[:], 1.0)
            jc = pool.tile([_P, _J], dt.bfloat16, tag="jc")
            # jc[p, k] = k + 1
            nc.vector.tensor_tensor_scan(
                jc[:], o16[:], z[:, 0:_J], 0.0, op.add, op.add
            )

            m = pool.tile([_P, _F], dt.float32, tag="m")
            nc.vector.tensor_scalar(m[:], xt[:], float(_THRES), None, op.is_le)
            L = pool.tile([_P, _F], dt.bfloat16, tag="L")
            nc.vector.tensor_tensor_scan(
                L[:], m[:], z[:], 0.0, op.add, op.add
            )

            # G[p, j, f] = (L[p, f] < jc[p, j]) = (L[p, f] <= j)
            G = pool.tile([_P, _J * _F], dt.bfloat16, tag="G")
            Lb = L[:].unsqueeze(1).broadcast_to((_P, _J, _F))
            jb = jc[:].unsqueeze(2).broadcast_to((_P, _J, _F))
            G3 = G[:].rearrange("p (j f) -> p j f", j=_J)
            nc.vector.tensor_tensor(G3, Lb, jb, op.is_lt)

            # CDF[p, j] = sum_f G[p, j, f]
            S = pool.tile([_P, _J], dt.bfloat16, tag="S")
            with nc.allow_low_precision(reason="counts <= 28 are exact in bf16"):
                nc.vector.tensor_reduce(
                    S[:], G3, axis=mybir.AxisListType.X, op=op.add
                )

            nc.sync.dma_start(cnt[:, :], S[:])
    nc.compile()
    return nc


def _get_nc():
    if "nc" not in _NC_CACHE:
        _NC_CACHE["nc"] = _build_nc()
    return _NC_CACHE["nc"]


def _decode_cdf(cdf):
    """cdf: [128, J] (bf16-ish floats) from one core ->
    positions [256] int64 in the 3584 prefix, or None if inconsistent."""
    c = np.asarray(cdf, dtype=np.float32)
    if not np.all(np.isfinite(c)):
        return None
    ci = c.astype(np.int64)
    if not np.array_equal(ci.astype(np.float32), c):
        return None
    if ci.min() < 0 or ci.max() > _F:
        return None
    if np.any(np.diff(ci, axis=1) < 0):
        return None
    t = (ci < _F).sum(axis=1)          # = min(t[p], J); exact iff t[p] < J
    if t.max() >= _J:
        return None
    if t.sum() < _K:
        return None
    base = np.concatenate([[0], np.cumsum(t)])
    r = np.arange(_K)
    p = np.searchsorted(base, r, side="right") - 1
    lr = r - base[p]
    pos = _F * p + ci[p, lr]
    if pos[0] < 0 or pos[-1] >= _PRE:
        return None
    if np.any(np.diff(pos) <= 0):
        return None
    return pos


def _run_device(prefix, trace=False):
    """prefix: [8, 3584] f32.  Returns (positions [8, 256] or None, results)."""
    from concourse.bass_utils import run_bass_kernel_spmd

    nc = _get_nc()
    in_maps = [
        {"x": np.ascontiguousarray(prefix[c].reshape(_P, _F))}
        for c in range(_NCORES)
    ]
    res = run_bass_kernel_spmd(
        nc, in_maps, core_ids=list(range(_NCORES)), trace=trace
    )
    pos = []
    for c in range(_NCORES):
        pc = _decode_cdf(res.results[c]["cnt"])
        if pc is None:
            return None, res
        pos.append(pc)
    return np.stack(pos), res


def _host_row(flat_row):
    """Exact reference semantics for one row (fallback path)."""
    mask = flat_row <= _THRES
    hits = np.flatnonzero(mask)
    if hits.size >= _K:
        return hits[:_K].astype(np.int64)
    masked = np.where(flat_row > _THRES, flat_row, np.float32(0.0))
    order = np.argsort(masked, kind="stable")
    return order[:_K].astype(np.int64)


def kernel(confidence_map):
    cm = np.asarray(confidence_map)
    if cm.dtype != np.float32:
        cm = cm.astype(np.float32)
    B = cm.shape[0]
    num_tgt = cm.shape[2]
    flat = cm.reshape(B, -1)

    idx = None
    if B == _NCORES and flat.shape[1] >= _PRE:
        idx, _ = _run_device(flat[:, :_PRE])
    if idx is None:
        idx = np.stack([_host_row(flat[b]) for b in range(B)])

    src = (idx // num_tgt).astype(np.int32)
    tgt = (idx % num_tgt).astype(np.int32)
    return np.stack([src, tgt], axis=-1)


# revision 3
# speedup vs baseline: 1.4688x; 1.0928x over previous
"""Trainium2 Bass kernel for NodeCorrespondenceSelector (topk_masking), v2.

Reference semantics: mask confidence <= 0.1 to zero, take the 256 SMALLEST
of the masked [B, N*M] map (top_k of the negation), unravel to (src, tgt).
~10% of uniform entries are <= 0.1 and become exactly 0.0, so the answer is
the first 256 flat indices with value <= 0.1 per batch row, ascending; all
of them live in a short prefix (256th hit ~ flat position 2600), so each
core scans a 3584-element prefix laid out [128 partitions x 28].

v2 device algorithm (per core, one batch row; everything stays in the
native [128, 28] layout -- no flatten DMAs, no matmuls, no PSUM):
  1. m = (x <= 0.1)                               [128, 28]
  2. L = inclusive cumsum of m along free dim     [128, 28]
  3. G[p, j, f] = (L[p, f] < j+1)  for j in 0..J-1, via stride-0
     broadcast APs (L broadcast along j, iota-by-scan jc broadcast
     along f)                                     [128, J*28]
  4. CDF[p, j] = sum_f G[p, j, f]  (tensor_reduce over the innermost
     axis of the 3D view)                         [128, J]
CDF[p, j] is the in-partition position of the (j+1)-th hit when
j < t[p] (t[p] = hits in partition p), else 28.  The host recovers
t[p] = #{j : CDF[p,j] < 28} (exact whenever t[p] < J, detectable
otherwise), builds the hit-count prefix sum over partitions, and decodes
rank r -> partition p(r) + local position CDF[p(r), r - base[p(r)]].
The host verifies the decode is consistent (integral CDF, nondecreasing
rows, t < J, >= 256 hits, strictly increasing positions) and falls back
to an exact host computation otherwise.
"""

import numpy as np

_THRES = np.float32(0.1)
_K = 256
_P = 128            # SBUF partitions
_F = 28             # free elements per partition in the prefix tile
_PRE = _P * _F      # 3584: prefix elements scanned on device per row
_J = 16             # CDF thresholds per partition (max decodable hits/partition)
_NCORES = 8

_NC_CACHE = {}


def _build_nc():
    import concourse.bacc as bacc
    import concourse.mybir as mybir
    from concourse.tile import TileContext

    dt = mybir.dt
    op = mybir.AluOpType

    nc = bacc.Bacc(trn_type="TRN2", debug=False, enable_asserts=False)
    x = nc.dram_tensor("x", [_P, _F], dt.float32, kind="ExternalInput")
    cnt = nc.dram_tensor("cnt", [_P, _J], dt.bfloat16, kind="ExternalOutput")

    with TileContext(nc) as tc:
        with tc.tile_pool(name="sb", bufs=1) as pool:
            xt = pool.tile([_P, _F], dt.float32, tag="xt")
            # single input DMA, issued from gpsimd (first engine free after
            # the block-entry barrier); splitting across queues makes the
            # last packet straggle ~2us
            nc.gpsimd.dma_start(xt[:], x[:, :])

            # constants built on DVE before the input lands
            z = pool.tile([_P, _F], dt.float32, tag="z")
            nc.vector.memset(z[:], 0.0)
            o16 = pool.tile([_P, _J], dt.float32, tag="o16")
            nc.vector.memset(o16[:], 1.0)
            jc = pool.tile([_P, _J], dt.bfloat16, tag="jc")
            # jc[p, k] = k + 1
            nc.vector.tensor_tensor_scan(
                jc[:], o16[:], z[:, 0:_J], 0.0, op.add, op.add
            )

            m = pool.tile([_P, _F], dt.float32, tag="m")
            nc.vector.tensor_scalar(m[:], xt[:], float(_THRES), None, op.is_le)
            L = pool.tile([_P, _F], dt.bfloat16, tag="L")
            nc.vector.tensor_tensor_scan(
                L[:], m[:], z[:], 0.0, op.add, op.add
            )

            # G[p, j, f] = (L[p, f] < jc[p, j]) = (L[p, f] <= j)
            G = pool.tile([_P, _J * _F], dt.bfloat16, tag="G")
            Lb = L[:].unsqueeze(1).broadcast_to((_P, _J, _F))
            jb = jc[:].unsqueeze(2).broadcast_to((_P, _J, _F))
            G3 = G[:].rearrange("p (j f) -> p j f", j=_J)
            nc.vector.tensor_tensor(G3, Lb, jb, op.is_lt)

            # CDF[p, j] = sum_f G[p, j, f]
            S = pool.tile([_P, _J], dt.bfloat16, tag="S")
            with nc.allow_low_precision(reason="counts <= 28 are exact in bf16"):
                nc.vector.tensor_reduce(
                    S[:], G3, axis=mybir.AxisListType.X, op=op.add
                )

            nc.sync.dma_start(cnt[:, :], S[:])
    nc.compile()
    return nc


def _get_nc():
    if "nc" not in _NC_CACHE:
        _NC_CACHE["nc"] = _build_nc()
    return _NC_CACHE["nc"]


def _decode_cdf(cdf):
    """cdf: [128, J] (bf16-ish floats) from one core ->
    positions [256] int64 in the 3584 prefix, or None if inconsistent."""
    c = np.asarray(cdf, dtype=np.float32)
    if not np.all(np.isfinite(c)):
        return None
    ci = c.astype(np.int64)
    if not np.array_equal(ci.astype(np.float32), c):
        return None
    if ci.min() < 0 or ci.max() > _F:
        return None
    if np.any(np.diff(ci, axis=1) < 0):
        return None
    t = (ci < _F).sum(axis=1)          # = min(t[p], J); exact iff t[p] < J
    if t.max() >= _J:
        return None
    if t.sum() < _K:
        return None
    base = np.concatenate([[0], np.cumsum(t)])
    r = np.arange(_K)
    p = np.searchsorted(base, r, side="right") - 1
    lr = r - base[p]
    pos = _F * p + ci[p, lr]
    if pos[0] < 0 or pos[-1] >= _PRE:
        return None
    if np.any(np.diff(pos) <= 0):
        return None
    return pos


def _run_device(prefix, trace=False):
    """prefix: [8, 3584] f32.  Returns (positions [8, 256] or None, results)."""
    from concourse.bass_utils import run_bass_kernel_spmd

    nc = _get_nc()
    in_maps = [
        {"x": np.ascontiguousarray(prefix[c].reshape(_P, _F))}
        for c in range(_NCORES)
    ]
    res = run_bass_kernel_spmd(
        nc, in_maps, core_ids=list(range(_NCORES)), trace=trace
    )
    pos = []
    for c in range(_NCORES):
        pc = _decode_cdf(res.results[c]["cnt"])
        if pc is None:
            return None, res
        pos.append(pc)
    return np.stack(pos), res


def _host_row(flat_row):
    """Exact reference semantics for one row (fallback path)."""
    mask = flat_row <= _THRES
    hits = np.flatnonzero(mask)
    if hits.size >= _K:
        return hits[:_K].astype(np.int64)
    masked = np.where(flat_row > _THRES, flat_row, np.float32(0.0))
    order = np.argsort(masked, kind="stable")
    return order[:_K].astype(np.int64)


def kernel(confidence_map):
    cm = np.asarray(confidence_map)
    if cm.dtype != np.float32:
        cm = cm.astype(np.float32)
    B = cm.shape[0]
    num_tgt = cm.shape[2]
    flat = cm.reshape(B, -1)

    idx = None
    if B == _NCORES and flat.shape[1] >= _PRE:
        idx, _ = _run_device(flat[:, :_PRE])
    if idx is None:
        idx = np.stack([_host_row(flat[b]) for b in range(B)])

    src = (idx // num_tgt).astype(np.int32)
    tgt = (idx % num_tgt).astype(np.int32)
    return np.stack([src, tgt], axis=-1)


# revision 5
# speedup vs baseline: 1.5327x; 1.0435x over previous
"""Trainium2 Bass kernel for NodeCorrespondenceSelector (topk_masking), v2.

Reference semantics: mask confidence <= 0.1 to zero, take the 256 SMALLEST
of the masked [B, N*M] map (top_k of the negation), unravel to (src, tgt).
~10% of uniform entries are <= 0.1 and become exactly 0.0, so the answer is
the first 256 flat indices with value <= 0.1 per batch row, ascending; all
of them live in a short prefix (256th hit ~ flat position 2600), so each
core scans a 3584-element prefix laid out [128 partitions x 28].

v2 device algorithm (per core, one batch row; everything stays in the
native [128, 28] layout -- no flatten DMAs, no matmuls, no PSUM):
  1. m = (x <= 0.1)                               [128, 28]
  2. L = inclusive cumsum of m along free dim     [128, 28]
  3. G[p, j, f] = (L[p, f] < j+1)  for j in 0..J-1, via stride-0
     broadcast APs (L broadcast along j, iota-by-scan jc broadcast
     along f)                                     [128, J*28]
  4. CDF[p, j] = sum_f G[p, j, f]  (tensor_reduce over the innermost
     axis of the 3D view)                         [128, J]
CDF[p, j] is the in-partition position of the (j+1)-th hit when
j < t[p] (t[p] = hits in partition p), else 28.  The host recovers
t[p] = #{j : CDF[p,j] < 28} (exact whenever t[p] < J, detectable
otherwise), builds the hit-count prefix sum over partitions, and decodes
rank r -> partition p(r) + local position CDF[p(r), r - base[p(r)]].
The host verifies the decode is consistent (integral CDF, nondecreasing
rows, t < J, >= 256 hits, strictly increasing positions) and falls back
to an exact host computation otherwise.
"""

import numpy as np

_THRES = np.float32(0.1)
_K = 256
_P = 128            # SBUF partitions
_F = 28             # free elements per partition in the prefix tile
_PRE = _P * _F      # 3584: prefix elements scanned on device per row
_J = 14             # CDF thresholds per partition (max decodable hits/partition)
_NCORES = 8

_NC_CACHE = {}


def _build_nc():
    import concourse.bacc as bacc
    import concourse.mybir as mybir
    from concourse.tile import TileContext

    dt = mybir.dt
    op = mybir.AluOpType

    nc = bacc.Bacc(trn_type="TRN2", debug=False, enable_asserts=False)
    x = nc.dram_tensor("x", [_P, _F], dt.float32, kind="ExternalInput")
    cnt = nc.dram_tensor("cnt", [_P, _J], dt.bfloat16, kind="ExternalOutput")

    with TileContext(nc) as tc:
        with tc.tile_pool(name="sb", bufs=1) as pool:
            xt = pool.tile([_P, _F], dt.float32, tag="xt")
            # single input DMA issued from sync (earliest engine into the
            # block after the entry barrier); splitting across queues makes
            # the last packet straggle ~2us
            nc.sync.dma_start(xt[:], x[:, :])

            # constants built on DVE before the input lands
            z = pool.tile([_P, _F], dt.float32, tag="z")
            nc.vector.memset(z[:], 0.0)
            o16 = pool.tile([_P, _J], dt.float32, tag="o16")
            nc.vector.memset(o16[:], 1.0)
            jc = pool.tile([_P, _J], dt.bfloat16, tag="jc")
            # jc[p, k] = k + 1
            nc.vector.tensor_tensor_scan(
                jc[:], o16[:], z[:, 0:_J], 0.0, op.add, op.add
            )

            m = pool.tile([_P, _F], dt.float32, tag="m")
            nc.vector.tensor_scalar(m[:], xt[:], float(_THRES), None, op.is_le)
            L = pool.tile([_P, _F], dt.bfloat16, tag="L")
            nc.vector.tensor_tensor_scan(
                L[:], m[:], z[:], 0.0, op.add, op.add
            )

            # G[p, j, f] = (L[p, f] < jc[p, j]) = (L[p, f] <= j)
            G = pool.tile([_P, _J * _F], dt.bfloat16, tag="G")
            Lb = L[:].unsqueeze(1).broadcast_to((_P, _J, _F))
            jb = jc[:].unsqueeze(2).broadcast_to((_P, _J, _F))
            G3 = G[:].rearrange("p (j f) -> p j f", j=_J)
            nc.vector.tensor_tensor(G3, Lb, jb, op.is_lt)

            # CDF[p, j] = sum_f G[p, j, f]
            S = pool.tile([_P, _J], dt.bfloat16, tag="S")
            with nc.allow_low_precision(reason="counts <= 28 are exact in bf16"):
                nc.vector.tensor_reduce(
                    S[:], G3, axis=mybir.AxisListType.X, op=op.add
                )

            nc.sync.dma_start(cnt[:, :], S[:])
    nc.compile()
    return nc


def _get_nc():
    if "nc" not in _NC_CACHE:
        _NC_CACHE["nc"] = _build_nc()
    return _NC_CACHE["nc"]


def _decode_cdf(cdf):
    """cdf: [128, J] (bf16-ish floats) from one core ->
    positions [256] int64 in the 3584 prefix, or None if inconsistent."""
    c = np.asarray(cdf, dtype=np.float32)
    if not np.all(np.isfinite(c)):
        return None
    ci = c.astype(np.int64)
    if not np.array_equal(ci.astype(np.float32), c):
        return None
    if ci.min() < 0 or ci.max() > _F:
        return None
    if np.any(np.diff(ci, axis=1) < 0):
        return None
    t = (ci < _F).sum(axis=1)          # = min(t[p], J); exact iff t[p] < J
    if t.max() >= _J:
        return None
    if t.sum() < _K:
        return None
    base = np.concatenate([[0], np.cumsum(t)])
    r = np.arange(_K)
    p = np.searchsorted(base, r, side="right") - 1
    lr = r - base[p]
    pos = _F * p + ci[p, lr]
    if pos[0] < 0 or pos[-1] >= _PRE:
        return None
    if np.any(np.diff(pos) <= 0):
        return None
    return pos


def _run_device(prefix, trace=False):
    """prefix: [8, 3584] f32.  Returns (positions [8, 256] or None, results)."""
    from concourse.bass_utils import run_bass_kernel_spmd

    nc = _get_nc()
    in_maps = [
        {"x": np.ascontiguousarray(prefix[c].reshape(_P, _F))}
        for c in range(_NCORES)
    ]
    res = run_bass_kernel_spmd(
        nc, in_maps, core_ids=list(range(_NCORES)), trace=trace
    )
    pos = []
    for c in range(_NCORES):
        pc = _decode_cdf(res.results[c]["cnt"])
        if pc is None:
            return None, res
        pos.append(pc)
    return np.stack(pos), res


def _host_row(flat_row):
    """Exact reference semantics for one row (fallback path)."""
    mask = flat_row <= _THRES
    hits = np.flatnonzero(mask)
    if hits.size >= _K:
        return hits[:_K].astype(np.int64)
    masked = np.where(flat_row > _THRES, flat_row, np.float32(0.0))
    order = np.argsort(masked, kind="stable")
    return order[:_K].astype(np.int64)


def kernel(confidence_map):
    cm = np.asarray(confidence_map)
    if cm.dtype != np.float32:
        cm = cm.astype(np.float32)
    B = cm.shape[0]
    num_tgt = cm.shape[2]
    flat = cm.reshape(B, -1)

    idx = None
    if B == _NCORES and flat.shape[1] >= _PRE:
        idx, _ = _run_device(flat[:, :_PRE])
    if idx is None:
        idx = np.stack([_host_row(flat[b]) for b in range(B)])

    src = (idx // num_tgt).astype(np.int32)
    tgt = (idx % num_tgt).astype(np.int32)
    return np.stack([src, tgt], axis=-1)
